# revision 1
# baseline (speedup 1.0000x reference)
"""AWQ 4-bit quantized linear (x @ dequant(qweight).T + bias) on 8 Trainium2 cores.

Column-parallel sharding: out_features (O=11008) split exactly across 8 cores
(1376 each = 10 full 128-row o-tiles + one 96-row tile — no pad, no wasted
matmul columns); x is replicated (fed transposed as xT so the contraction dim lands on
SBUF partitions). Each core dequantizes its weight shard on-device into SBUF
([I, O_sh] fp16, ~11.5 MB resident) and streams x tiles through the PE.

  kernel(x, qweight, qzeros, scales, bias) -> [8192, 11008] fp16
"""

import numpy as np
from contextlib import ExitStack

import concourse.bacc as bacc
import concourse.mybir as mybir
import concourse.tile as tile
from concourse._compat import with_exitstack
from concourse.bass_utils import run_bass_kernel_spmd
from concourse.masks import make_identity


class _Bacc(bacc.Bacc):
    """Bacc that keeps matmuls self-loading.

    The stock `move_matmul_waits_to_ldweights` pass splits every InstMatmult
    into an explicit InstLdweights + InstMatmult; explicit LDWEIGHTS skips
    walrus's fast-weight-load codegen and measured ~117ns per matmul (~45ns
    un-hidden PE stall each). Self-loading matmuls let walrus emit the
    optimized weight load. Extra semaphore waits that the pass would have
    parked on the LDWEIGHTS are handled by `generate_event_semaphores`.
    """

    def move_matmul_waits_to_ldweights(self):
        pass

PACK = 8      # int32 packs 8 x 4-bit values, low nibble first
QBIT = 4
GS = 128      # quant group size == matmul k-tile size
NCORES = 8
TCH = 256     # t-columns fetched per x-tile DMA (2 PSUM t-tiles)

f16 = mybir.dt.float16
i16 = mybir.dt.int16
i32 = mybir.dt.int32
f32 = mybir.dt.float32
LSR = mybir.AluOpType.logical_shift_right
AND = mybir.AluOpType.bitwise_and
SUB = mybir.AluOpType.subtract
MUL = mybir.AluOpType.mult
ADD = mybir.AluOpType.add


def _n_splits(o_sh):
    # largest-first: a 512,512,384 pattern per k-step measured 216ns/MM
    # steady-state pace; 384-first measured 259ns (LDW overlap interaction)
    splits, off = [], 0
    while off < o_sh:
        n = min(512, o_sh - off)
        splits.append((off, n))
        off += n
    return splits


@with_exitstack
def _emit(ctx, tc, T, I, O_SH, xT, qw, qz, sc, b, out):
    nc = tc.nc
    KT = I // 128          # k-tiles (== quant groups, since GS == 128)
    NG = I // GS
    OT = -(-O_SH // 128)   # o-tiles per shard (last may be partial)
    assert I % (128 * PACK) == 0 and T % TCH == 0 and O_SH % 16 == 0
    assert NG % PACK == 0

    const_pool = ctx.enter_context(tc.tile_pool(name="const", bufs=1))
    wt_pool = ctx.enter_context(tc.tile_pool(name="wt", bufs=1))
    deq_pool = ctx.enter_context(tc.tile_pool(name="deq", bufs=2))
    x_pool = ctx.enter_context(tc.tile_pool(name="x", bufs=3))
    o_pool = ctx.enter_context(tc.tile_pool(name="o", bufs=2))
    psT_pool = ctx.enter_context(tc.tile_pool(name="psT", bufs=2, space="PSUM"))
    ps_pool = ctx.enter_context(tc.tile_pool(name="ps", bufs=2, space="PSUM"))

    bias_bc = const_pool.tile([128, O_SH], f16)
    nc.sync.dma_start(bias_bc[:], b.broadcast_to([128, O_SH]))
    ident = const_pool.tile([128, 128], f16)
    make_identity(nc, ident[:])

    # Resident dequantized, transposed weights: [128 (i in k-tile), KT, O_SH]
    WT = wt_pool.tile([128, KT, O_SH], f16)

    # ---- Phase A: dequantize the shard ----
    def qwt_load(j):
        rj = min(128, O_SH - j * 128)
        t = deq_pool.tile([128, I // PACK], i32, tag="qwt", name="qwt")
        nc.sync.dma_start(t[:rj, :], qw[j * 128 : j * 128 + rj, :])
        return t

    # qweight for j0 first in the DMA queue: its unpack is the head of the
    # startup critical path and mustn't wait behind the 22 z/s prep DMAs
    qwt_pre = {0: qwt_load(0)}

    # Zero/scale prep batched across ALL o-tiles up front: one [128, OT*NG]
    # tile per quantity instead of 11 sets of tiny per-o-tile ops (~18us of
    # DVE op overhead saved, and it removes the per-j dependency chains).
    zq_all = const_pool.tile([128, OT, NG // PACK], i32)
    sc_all = const_pool.tile([128, OT, NG], f16)
    nc.gpsimd.memset(zq_all[:], 0)
    nc.gpsimd.memset(sc_all[:], 0.0)
    for j in range(OT):
        rj = min(128, O_SH - j * 128)
        js = slice(j * 128, j * 128 + rj)
        nc.sync.dma_start(zq_all[:rj, j, :], qz[js, :])
        nc.sync.dma_start(sc_all[:rj, j, :], sc[js, :])
    zi_all = const_pool.tile([128, OT, NG], i32)
    for k in range(PACK):
        nc.vector.tensor_scalar(
            zi_all.rearrange("p o (c k) -> p o k c", k=PACK)[:, :, k, :],
            zq_all[:], QBIT * k, 0xF, LSR, AND,
        )
    zf_all = const_pool.tile([128, OT, NG], f32)
    nc.vector.tensor_copy(zf_all[:], zi_all[:])
    sf_all = const_pool.tile([128, OT, NG], f32)
    nc.vector.tensor_copy(sf_all[:], sc_all[:])
    # nzs = -z*s so the affine can run on ACT as activation(v*s + (-z*s))
    nzs_all = const_pool.tile([128, OT, NG], f32)
    nc.vector.scalar_tensor_tensor(nzs_all[:], zf_all[:], -1.0, sf_all[:], MUL, MUL)

    # o-tile order: j0-3 first (unlock the first 512-wide n-split), then the
    # LAST n-split's o-tiles (only 3 of them — unlocks the 384-wide chains as
    # extra early PE work while the middle o-tiles still dequantize), then
    # the middle ones. Steady-state per-k MM emission order is unchanged.
    j_order = (
        list(range(min(4, OT)))
        + list(range(8, OT))
        + list(range(4, min(8, OT)))
    )
    for j in j_order:
        rj = min(128, O_SH - j * 128)   # rows in this o-tile (last may be 96)
        js = slice(j * 128, j * 128 + rj)
        qwt = qwt_pre.pop(j) if j in qwt_pre else qwt_load(j)

        # bitvec ops can't cast, so unpack int32->int32 and cast in later ops
        uq = deq_pool.tile([128, I], i32, tag="uq")
        for k in range(PACK):
            nc.vector.tensor_scalar(
                uq[:rj, k::PACK], qwt[:rj, :], QBIT * k, 0xF, LSR, AND
            )

        # dequant affine, split 1/4 DVE : 3/4 ACT to balance the two engines'
        # phase-A load (DVE also carries the unpacks and j<4 psum copies;
        # ACT ops measured ~0.49us vs DVE ~0.34us for these [128,128] tiles)
        wq = deq_pool.tile([128, I], f16, tag="wq")
        for g in range(NG):
            gs = slice(g * GS, (g + 1) * GS)
            if g % 4 == 0:
                nc.vector.tensor_scalar(
                    wq[:rj, gs], uq[:rj, gs],
                    zf_all[:rj, j, g : g + 1], sf_all[:rj, j, g : g + 1],
                    SUB, MUL,
                )
            else:
                nc.scalar.activation(
                    wq[:rj, gs], uq[:rj, gs],
                    mybir.ActivationFunctionType.Identity,
                    bias=nzs_all[:rj, j, g : g + 1],
                    scale=sf_all[:rj, j, g : g + 1],
                )

        # Transpose into WT's k-chunk layout. For the first o-tiles (which
        # gate the first accumulation chains) use fine-grained PE transposes
        # (low latency); for the rest use one whole-tile xbar DMA each (off
        # the PE, and few enough that xbar<->copy mode serialization stays
        # off the critical path). 4 o-tiles cover the first 512-wide n-split.
        if j < 4:
            for g in range(KT):
                pst = psT_pool.tile([128, 128], f16, tag="pst")
                nc.tensor.transpose(
                    pst[:, :rj], wq[:rj, g * 128 : (g + 1) * 128], ident[:rj, :rj]
                )
                nc.vector.tensor_copy(WT[:, g, js], pst[:, :rj])
        else:
            # WT[p, g, js+f] = wq[f, g*128+p]
            nc.sync.dma_start_transpose(WT[:, :, js], wq[:rj, :])

    # ---- Phase B: stream x through the PE ----
    splits = _n_splits(O_SH)
    xT_r = xT.rearrange("(k p) t -> p k t", p=128)  # [128, KT, T]

    def chains(ti, use_splits):
        xt = x_pool.tile([128, KT, TCH], f16, tag="xt", name="xt")
        nc.sync.dma_start(xt[:], xT_r[:, :, ti * TCH : (ti + 1) * TCH])
        for h in range(TCH // 128):
            tsl = slice(h * 128, (h + 1) * 128)
            psums = [
                ps_pool.tile([128, nsz], f32, tag=f"ps{noff}", name=f"ps{noff}")
                for noff, nsz in use_splits
            ]
            for k in range(KT):
                for ps, (noff, nsz) in zip(psums, use_splits):
                    nc.tensor.matmul(
                        ps[:],
                        xt[:, k, tsl],
                        WT[:, k, noff : noff + nsz],
                        start=(k == 0),
                        stop=(k == KT - 1),
                    )
            # per-n-chain epilogue + store, so each PSUM slot recycles as soon
            # as its own chain finishes (no coupling across the chains)
            t0 = ti * TCH + h * 128
            for ps, (noff, nsz) in zip(psums, use_splits):
                ot = o_pool.tile([128, nsz], f16, tag=f"ot{noff}", name=f"ot{noff}")
                nc.vector.tensor_tensor(
                    ot[:], ps[:], bias_bc[:, noff : noff + nsz], ADD
                )
                nc.sync.dma_start(out[t0 : t0 + 128, noff : noff + nsz], ot[:])

    for ti in range(T // TCH):
        chains(ti, splits)


def _build(T, I, O_SH):
    nc = _Bacc(
        "TRN2",
        target_bir_lowering=False,
        debug=False,
        enable_asserts=False,
        num_devices=NCORES,
    )
    xT_d = nc.dram_tensor("xT", [I, T], f16, kind="ExternalInput")
    qw_d = nc.dram_tensor("qw", [O_SH, I // PACK], i32, kind="ExternalInput")
    qz_d = nc.dram_tensor("qz", [O_SH, I // GS // PACK], i32, kind="ExternalInput")
    sc_d = nc.dram_tensor("sc", [O_SH, I // GS], f16, kind="ExternalInput")
    b_d = nc.dram_tensor("b", [1, O_SH], f16, kind="ExternalInput")
    out_d = nc.dram_tensor("out", [T, O_SH], f16, kind="ExternalOutput")
    with tile.TileContext(nc) as tc:
        _emit(
            tc, T, I, O_SH,
            xT_d.ap(), qw_d.ap(), qz_d.ap(), sc_d.ap(), b_d.ap(), out_d.ap(),
        )
    nc.compile()
    return nc


_NC_CACHE = {}


def _get_nc(T, I, O_SH):
    key = (T, I, O_SH)
    if key not in _NC_CACHE:
        _NC_CACHE[key] = _build(*key)
    return _NC_CACHE[key]


def _shard_inputs(x, qweight, qzeros, scales, bias):
    T, I = x.shape
    O = qweight.shape[0]
    o_pad = -(-O // (16 * NCORES)) * (16 * NCORES)
    o_sh = o_pad // NCORES
    xT = np.ascontiguousarray(x.T)

    def pad_rows(a):
        if a.shape[0] == o_pad:
            return a
        pad = np.zeros((o_pad - a.shape[0],) + a.shape[1:], a.dtype)
        return np.concatenate([a, pad], axis=0)

    qw_p = pad_rows(np.asarray(qweight))
    qz_p = pad_rows(np.asarray(qzeros))
    sc_p = pad_rows(np.asarray(scales))
    b_p = pad_rows(np.asarray(bias))
    in_maps = []
    for c in range(NCORES):
        rows = slice(c * o_sh, (c + 1) * o_sh)
        in_maps.append(
            {
                "xT": xT,
                "qw": np.ascontiguousarray(qw_p[rows]),
                "qz": np.ascontiguousarray(qz_p[rows]),
                "sc": np.ascontiguousarray(sc_p[rows]),
                "b": np.ascontiguousarray(b_p[rows]).reshape(1, o_sh),
            }
        )
    return in_maps, T, I, O, o_sh


def _run(x, qweight, qzeros, scales, bias, trace=False, **kw):
    in_maps, T, I, O, o_sh = _shard_inputs(x, qweight, qzeros, scales, bias)
    nc = _get_nc(T, I, o_sh)
    res = run_bass_kernel_spmd(nc, in_maps, list(range(NCORES)), trace=trace, **kw)
    out = np.concatenate([res.results[c]["out"] for c in range(NCORES)], axis=1)
    return out[:, :O], res


def kernel(x, qweight, qzeros, scales, bias):
    out, _ = _run(x, qweight, qzeros, scales, bias)
    return out

